# revision 1
# baseline (speedup 1.0000x reference)
"""ConvexSoftMixer Trainium2 kernel.

Shards batch*heads (1*8 = 8) across 8 NeuronCores, one head per core.

Math (exact refactor of the reference; m1 cancels analytically):
    f_q[s] = sum_j softplus(softplus(q @ spW1q.T + b1) @ spW2q.T + b2)[s,j]
    g_k[t] likewise for k
    phi_q = exp(q @ Wh.T); phi_k = exp(k @ Wh.T); u = v @ Wv.T
    c[t,p]  = g_k[t] - log(S) + u[t,p]
    m2[p]   = max_t c[t,p]
    E[t,p]  = exp(c[t,p] - m2[p])
    M[r,p]  = sum_t phi_k[t,r] * E[t,p]
    y[s,p]  = f_q[s] + m2[p] + log( sum_r phi_q[s,r] * M[r,p] )
(The -log(S) is folded into g_k: it shifts m2 by -log(S) and cancels in E.)

On-device layout is transposed (feature dim on SBUF partitions, sequence on
the free dim) so the ICNN layers chain as matmuls with no transposes. q and
k ICNNs are stacked on 128 partitions with block-diagonal weights. All
partition-dim broadcasts are done as rank-1 matmul accumulations into PSUM
using constant rows packed into the host-prepared input tensors.
"""

import math

import numpy as np

_B, _H, _S, _D, _P = 1, 8, 512, 64, 32
_NCORES = 8
_LN_S = math.log(float(_S))

_CACHE = {}


def _build_bass(dump=False):
    import concourse.tile as tile
    from concourse import bacc, mybir

    f32 = mybir.dt.float32
    AF = mybir.ActivationFunctionType
    AX = mybir.AxisListType.X

    # Bacc (not raw Bass): its compile passes split multi-sem waits (TRN2
    # allows one wait per instruction) and insert ACT table loads.
    nc = bacc.Bacc("TRN2", target_bir_lowering=False, debug=False)

    # DRAM I/O (per core). Read-only inputs ride in ONE tensor/DMA; column map:
    # [0:512) xqk | [512:643) w1b | [643:772) w2b | [772:836) whv (rows 0-65)
    # | [836:1348) kt (rows 0-63).  vta is separate because the device writes
    # g_k into its row 64 (tile-granular deps stay exact that way).
    _MW = 1348
    mega_d = nc.dram_tensor("mega", [128, _MW], f32, kind="ExternalInput").ap()
    vta_d = nc.dram_tensor("vta", [_D + 2, _S], f32, kind="ExternalInput").ap()
    misc_d = nc.dram_tensor("misc", [1, 128 + _S], f32, kind="ExternalInput").ap()
    y_d = nc.dram_tensor("y", [_P, _S], f32, kind="ExternalOutput").ap()

    NCH = _S // 128  # 4 sequence chunks of 128 for [t, p]-layout stages

    with tile.TileContext(nc) as tc:
        with (
            tc.tile_pool(name="pin", bufs=1) as pin,
            tc.tile_pool(name="pwork", bufs=1) as pw,
            # PSUM: tags share slots; lifetimes are disjoint within a tag.
            tc.tile_pool(name="psA", bufs=2, space="PSUM") as psA,  # z1,z2 / AT,F
            tc.tile_pool(name="psB", bufs=2, space="PSUM") as psB,  # gk,cT / phiq,M
            tc.tile_pool(name="psC", bufs=2, space="PSUM") as psC,  # pk, ec
            tc.tile_pool(name="psD", bufs=1, space="PSUM") as psD,  # fq
        ):
            # ---- input loads ----
            mega = pin.tile([128, _MW], f32, tag="mega")
            nc.sync.dma_start(out=mega, in_=mega_d)
            vta = pin.tile([_D + 2, _S], f32, tag="vta")
            nc.sync.dma_start(out=vta, in_=vta_d)
            misc = pin.tile([1, 128 + _S], f32, tag="misc")
            nc.sync.dma_start(out=misc, in_=misc_d)

            xqk = mega[:, 0:512]
            w1b = mega[:, 512:643]
            w2b = mega[:, 643:772]
            whv = mega[0:_D + 2, 772:836]
            kt = mega[0:_D, 836:1348]

            # named slices of the packed inputs
            w1 = w1b[:, 0:128]        # block-diag softplus'd layer-1 weights (T)
            b1 = w1b[:, 128:129]      # stacked layer-1 bias column
            eq = w1b[:, 129:130]      # [1]*64 + [0]*64 column
            ek = w1b[:, 130:131]      # [0]*64 + [1]*64 column
            w2 = w2b[:, 0:128]
            wv_aug = whv[:, 0:_P]     # rows 0-63 Wv.T, row 64 = 1.0, row 65 = 0
            wh_t = whv[0:_D, _P:2 * _P]  # Wh.T
            b2row = misc[0:1, 0:128]  # layer-2 bias as a [1, 128] row
            ones_row = misc[0:1, 128:128 + _S]  # [1, S] of 1.0

            # ---- stacked ICNN (q rows 0-63, k rows 64-127) ----
            z1_p = psA.tile([128, _S], f32, tag="big")
            nc.tensor.matmul(out=z1_p, lhsT=w1, rhs=xqk, start=True, stop=True)
            e1 = pw.tile([128, _S], f32, tag="e1")
            nc.scalar.activation(out=e1, in_=z1_p, func=AF.Exp, bias=b1, scale=1.0)
            z1 = pw.tile([128, _S], f32, tag="z1")
            nc.scalar.activation(out=z1, in_=e1, func=AF.Ln, bias=1.0, scale=1.0)

            # layer-2 args can exceed the Exp LUT's input clamp (~41), so:
            # softplus(x) = max(x, ln(1 + exp(min(x, 30))))   (exact in f32:
            # for x > 30, softplus(x) == x and softplus >= x always).
            z2_p = psA.tile([128, _S], f32, tag="big")
            nc.tensor.matmul(out=z2_p, lhsT=w2, rhs=z1, start=True, stop=False)
            nc.tensor.matmul(out=z2_p, lhsT=b2row, rhs=ones_row,
                             start=False, stop=True)  # + b2 broadcast
            z2c = pw.tile([128, _S], f32, tag="z2c")
            nc.vector.tensor_scalar_min(z2c, z2_p, 30.0)
            e2 = pw.tile([128, _S], f32, tag="e2")
            nc.scalar.activation(out=e2, in_=z2c, func=AF.Exp, bias=0.0, scale=1.0)
            l2 = pw.tile([128, _S], f32, tag="l2")
            nc.scalar.activation(out=l2, in_=e2, func=AF.Ln, bias=1.0, scale=1.0)
            z2 = pw.tile([128, _S], f32, tag="z2")
            nc.vector.tensor_max(out=z2, in0=l2, in1=z2_p)

            # ---- phi_k chunks [t,r] (independent of ICNN; overlaps) ----
            pk_p = psC.tile([128, NCH * _P], f32, tag="chunk")
            for c in range(NCH):
                nc.tensor.matmul(
                    out=pk_p[:, c * _P:(c + 1) * _P],
                    lhsT=kt[:, c * 128:(c + 1) * 128],
                    rhs=wh_t,
                    start=True, stop=True,
                )
            pk = pw.tile([128, NCH * _P], f32, tag="pk")
            nc.scalar.activation(out=pk, in_=pk_p, func=AF.Exp, bias=0.0, scale=1.0)

            # ---- phi_q [r, s] ----
            phiq_p = psB.tile([_P, _S], f32, tag="mid")
            nc.tensor.matmul(out=phiq_p, lhsT=wh_t, rhs=xqk[0:_D, :], start=True, stop=True)
            phiq = pw.tile([_P, _S], f32, tag="phiq")
            nc.scalar.activation(out=phiq, in_=phiq_p, func=AF.Exp, bias=0.0, scale=1.0)

            # ---- f_q, g_k row sums of z2 (masked ones matmuls) ----
            fq_p = psD.tile([1, _S], f32, tag="fq")
            nc.tensor.matmul(out=fq_p, lhsT=eq, rhs=z2, start=True, stop=True)
            gk_p = psB.tile([1, _S], f32, tag="mid")
            nc.tensor.matmul(out=gk_p, lhsT=ek, rhs=z2, start=True, stop=True)

            fq = pw.tile([1, _S], f32, tag="fq_sb")
            nc.vector.tensor_copy(out=fq, in_=fq_p)
            # g_k - log(S) written into vta row 64 (pairs with wv_aug's 1.0 row)
            nc.vector.tensor_scalar_add(vta[_D:_D + 1, :], gk_p, -_LN_S)

            # ---- cT[p,t] = u.T + g_k broadcast ; m2 = rowmax ----
            cT_p = psB.tile([_P, _S], f32, tag="mid")
            nc.tensor.matmul(
                out=cT_p, lhsT=wv_aug[0:_D + 1, :], rhs=vta[0:_D + 1, :],
                start=True, stop=True,
            )
            m2pad = pw.tile([_P, _P], f32, tag="m2pad")
            nc.vector.memset(m2pad, 0.0)
            nc.vector.reduce_max(m2pad[:, 0:1], cT_p, axis=AX)
            m2t = pw.tile([_P, _P], f32, tag="m2t")
            nc.vector.transpose(m2t, m2pad)  # row 0 of m2t = m2 as [1, P]
            negm2_4 = pw.tile([1, NCH * _P], f32, tag="negm2")
            for c in range(NCH):
                nc.vector.tensor_scalar_mul(
                    negm2_4[0:1, c * _P:(c + 1) * _P], m2t[0:1, 0:_P], -1.0)

            # ---- E chunks [t,p] = exp(u + g_k - m2) ----
            # u + g_k via the augmented matmul (vta rows 64=g_k, 65=1.0 paired
            # with wv_aug rows 64=1.0, 65=0), then one rank-1 matmul adds the
            # tiled -m2 row across all four chunks at once.
            ec_p = psC.tile([128, NCH * _P], f32, tag="chunk")
            for c in range(NCH):
                nc.tensor.matmul(
                    out=ec_p[:, c * _P:(c + 1) * _P],
                    lhsT=vta[:, c * 128:(c + 1) * 128],
                    rhs=wv_aug,
                    start=True, stop=False,
                )
                nc.tensor.matmul(
                    out=ec_p[:, c * _P:(c + 1) * _P],
                    lhsT=ones_row[0:1, c * 128:(c + 1) * 128],
                    rhs=negm2_4[0:1, 0:_P],
                    start=False, stop=True,
                )
            ec = pw.tile([128, NCH * _P], f32, tag="ec")
            nc.scalar.activation(out=ec, in_=ec_p, func=AF.Exp, bias=0.0, scale=1.0)

            # ---- M[r,p] = sum_t phi_k E ----
            M_p = psB.tile([_P, _P], f32, tag="mid")
            for c in range(NCH):
                nc.tensor.matmul(
                    out=M_p,
                    lhsT=pk[:, c * _P:(c + 1) * _P],
                    rhs=ec[:, c * _P:(c + 1) * _P],
                    start=(c == 0), stop=(c == NCH - 1),
                )
            M_sb = pw.tile([_P, _P], f32, tag="M_sb")
            nc.vector.tensor_copy(out=M_sb, in_=M_p)

            # ---- A.T = M.T-style matmul; y ----
            at_p = psA.tile([_P, _S], f32, tag="big")
            nc.tensor.matmul(out=at_p, lhsT=M_sb, rhs=phiq, start=True, stop=True)

            # F[p,s] = f_q[s] + m2[p] (two rank-1 broadcasts)
            f_p = psA.tile([_P, _S], f32, tag="big")
            nc.tensor.matmul(out=f_p, lhsT=ones_row[0:1, 0:_P], rhs=fq,
                             start=True, stop=False)
            nc.tensor.matmul(out=f_p, lhsT=m2t[0:1, 0:_P], rhs=ones_row,
                             start=False, stop=True)

            lnA = pw.tile([_P, _S], f32, tag="lnA")
            nc.scalar.activation(out=lnA, in_=at_p, func=AF.Ln, bias=0.0, scale=1.0)
            yT = pw.tile([_P, _S], f32, tag="yT")
            nc.vector.tensor_add(out=yT, in0=lnA, in1=f_p)

            nc.sync.dma_start(out=y_d, in_=yT)

            if dump:
                for nm, t in [
                    ("d_z1", z1), ("d_z2", z2), ("d_fq", fq), ("d_pk", pk),
                    ("d_ec", ec), ("d_phiq", phiq), ("d_m2t", m2t),
                    ("d_Msb", M_sb), ("d_lnA", lnA), ("d_vta64", vta[_D:_D + 1, :]),
                    ("d_negm2", negm2_4[0:1, 0:_P]),
                ]:
                    dd = nc.dram_tensor(nm, list(t.shape), f32,
                                        kind="ExternalOutput").ap()
                    nc.sync.dma_start(out=dd, in_=t)

    if not nc.is_finalized():
        nc.finalize()  # runs Bacc passes (wait splitting, reg alloc, ACT table loads)
    return nc


def _host_inputs(q, k, v, spW1q, b1q, spW2q, b2q, spW1k, b1k, spW2k, b2k, Wh, Wv):
    """Build the per-core input maps (numpy layout prep only)."""
    S, D, P = _S, _D, _P
    z = np.zeros
    # block-diagonal transposed weights + packed bias/mask columns (shared)
    w1b = z((128, 131), np.float32)
    w1b[0:D, 0:D] = spW1q.T
    w1b[D:2 * D, D:2 * D] = spW1k.T
    w1b[0:D, 128] = b1q
    w1b[D:2 * D, 128] = b1k
    w1b[0:D, 129] = 1.0     # eq
    w1b[D:2 * D, 130] = 1.0  # ek
    w2b = z((128, 129), np.float32)
    w2b[0:D, 0:D] = spW2q.T
    w2b[D:2 * D, D:2 * D] = spW2k.T
    w2b[0:D, 128] = b2q
    w2b[D:2 * D, 128] = b2k
    whv = z((D + 2, 2 * P), np.float32)
    whv[0:D, 0:P] = Wv.T
    whv[D, 0:P] = 1.0       # pairs with the g_k row of vta
    whv[0:D, P:2 * P] = Wh.T
    misc = z((1, 128 + S), np.float32)
    misc[0, 0:D] = b2q
    misc[0, D:128] = b2k
    misc[0, 128:] = 1.0

    in_maps = []
    for h in range(_H):
        qT = np.ascontiguousarray(q[0, h].T)
        kT = np.ascontiguousarray(k[0, h].T)
        vT = v[0, h].T
        mega = z((128, 1348), np.float32)
        mega[0:D, 0:S] = qT
        mega[D:2 * D, 0:S] = kT
        mega[:, 512:643] = w1b
        mega[:, 643:772] = w2b
        mega[0:D + 2, 772:836] = whv
        mega[0:D, 836:1348] = kT
        vta = z((D + 2, S), np.float32)
        vta[0:D] = vT
        # row D gets g_k - log(S) on device; row D+1 is constant ones
        vta[D + 1] = 1.0
        in_maps.append(dict(mega=mega, vta=vta, misc=misc))
    return in_maps


def kernel(**inputs):
    from concourse.bass_utils import run_bass_kernel_spmd

    np_in = {k: np.asarray(v) for k, v in inputs.items()}
    q, k, v = np_in["q"], np_in["k"], np_in["v"]

    def sp(x):  # softplus for the small weight matrices (host prep)
        return np.log1p(np.exp(x.astype(np.float64))).astype(np.float32)

    in_maps = _host_inputs(
        q, k, v,
        sp(np_in["sq_raw1"]), np_in["sq_b1"], sp(np_in["sq_raw2"]), np_in["sq_b2"],
        sp(np_in["sk_raw1"]), np_in["sk_b1"], sp(np_in["sk_raw2"]), np_in["sk_b2"],
        np_in["Wh"], np_in["Wv"],
    )

    if "nc" not in _CACHE:
        _CACHE["nc"] = _build_bass()
    nc = _CACHE["nc"]

    res = run_bass_kernel_spmd(nc, in_maps, list(range(_NCORES)))
    out = np.zeros((_B, _H, _S, _P), np.float32)
    for h in range(_H):
        out[0, h] = res.results[h]["y"].T
    return out



# revision 23
# speedup vs baseline: 1.8477x; 1.8477x over previous
"""ConvexSoftMixer Trainium2 kernel.

Shards batch*heads (1*8 = 8) across 8 NeuronCores, one head per core.

Math (exact refactor of the reference; m1 cancels analytically):
    f_q[s] = sum_j softplus(softplus(q @ spW1q.T + b1) @ spW2q.T + b2)[s,j]
    g_k[t] likewise for k
    phi_q = exp(q @ Wh.T); phi_k = exp(k @ Wh.T); u = v @ Wv.T
    c[t,p]  = g_k[t] + u[t,p]
    m2[p]   = max_t c[t,p]
    E[t,p]  = exp(c[t,p] - m2[p])
    M[r,p]  = sum_t phi_k[t,r] * E[t,p]
    y[s,p]  = f_q[s] + m2[p] + log( sum_r phi_q[s,r] * M[r,p] ) + delta
delta = sum(b2q) + sum(b2k) - log(S) is a pure additive output shift
(constant-in-t terms of g_k pass through max/exp/log unchanged), applied
on the host after gather.

Performance structure:
- All matmul operands are bf16 (1 PE cycle/row vs 4 for fp32; half the
  input DMA bytes). PSUM accumulation stays fp32. |y| ~ 6e3 vs a 2e-2
  relative gate, so bf16 quantization (~1e-3 rel) is far inside budget.
- Layer-2 softplus is clamp-free: x + ln(1+exp(-x)) with -b2 folded
  into the Exp bias column; only the f/g row sums of layer 2 are needed,
  so the linear part comes from one matmul against host-precomputed
  column sums of spW2 and no z2 tensor is materialized.
- All activations are Exp/Ln, forced onto the one ACT table holding
  both (natural_log_exp_and_others) so the 1283ns table load happens
  once, against a warm-up dummy activation that overlaps the input DMA.
- -m2 is written into row 65 of a device-side copy of the Wv weights,
  so each E chunk is a single matmul (vta row 65 is constant 1.0).
- The final y = ln(A) + f_q + m2 uses one rank-1 matmul for the f_q
  broadcast and a fused DVE scalar_tensor_tensor for the m2 column.
"""

import math

import numpy as np

_B, _H, _S, _D, _P = 1, 8, 512, 64, 32
_NCORES = 8
_LN_S = math.log(float(_S))

_CACHE = {}

# wt column map (bf16 weight tile). wsum/eqek blocks are 33 wide with the
# k-column at offset 32 so the fg matmul writes g_k to PSUM partition 32
# (engine APs must start at a multiple-of-32 partition).
_W1 = 0
_W2 = 128
_WSUM, _EQEK = 256, 289
_WV, _WH = 322, 354
_WT_COLS = 386


def _build_bass(dump=False):
    import bass_rust as _bass_rust
    import concourse.tile as tile
    from concourse import bacc, mybir
    from concourse.hw_specs import get_activation_tables

    f32 = mybir.dt.float32
    bf16 = mybir.dt.bfloat16
    AF = mybir.ActivationFunctionType
    AX = mybir.AxisListType.X
    ALU = None

    nc = bacc.Bacc("TRN2", target_bir_lowering=False, debug=False)

    # All activations here are Exp/Ln; both live in the
    # natural_log_exp_and_others table. The stock ATL pass picks the
    # first table per function (exp->0, ln->5) and thrashes 1283ns
    # reloads on every switch. Hand it a table list (same names/order,
    # so emitted act_func_set_ids still index the real act_info.json)
    # where only the shared table advertises Exp/Ln.
    tabs = get_activation_tables(nc.m.arch)
    doctored = []
    for name, funcs in tabs.items():
        if name != "natural_log_exp_and_others":
            funcs = funcs - {AF.Exp, AF.Ln}
        doctored.append((name, funcs))
    nc.insert_act_table_loads = lambda: _bass_rust.insert_act_table_loads(
        nc, doctored
    )

    wt_d = nc.dram_tensor("wt", [128, _WT_COLS], bf16, kind="ExternalInput").ap()
    colf_d = nc.dram_tensor("colf", [128, 2], f32, kind="ExternalInput").ap()
    xqk_d = nc.dram_tensor("xqk", [128, _S], bf16, kind="ExternalInput").ap()
    vta_d = nc.dram_tensor("vta", [_D + 1, _S], bf16, kind="ExternalInput").ap()
    misc_d = nc.dram_tensor("misc", [1, 128], bf16, kind="ExternalInput").ap()
    y_d = nc.dram_tensor("y", [_P, _S], f32, kind="ExternalOutput").ap()

    NCH = _S // 128  # 4 sequence chunks of 128 for [t, p]-layout stages

    with tile.TileContext(nc) as tc:
        with (
            tc.tile_pool(name="pin", bufs=1) as pin,
            tc.tile_pool(name="pwork", bufs=1) as pw,
            tc.tile_pool(name="psA", bufs=2, space="PSUM") as psA,  # z1,z2/f,at
            tc.tile_pool(name="psB", bufs=2, space="PSUM") as psB,  # fg, cT
            tc.tile_pool(name="psC", bufs=2, space="PSUM") as psC,  # pk, ec
            tc.tile_pool(name="psD", bufs=1, space="PSUM") as psD,  # M
            tc.tile_pool(name="psE", bufs=1, space="PSUM") as psE,  # phiq
        ):
            # ---- ACT table warm-up: no DMA deps, runs during input DMA ----
            scr = pw.tile([1, 2], f32, tag="scr")
            nc.gpsimd.memset(scr, 0.0)
            scr2 = pw.tile([1, 2], f32, tag="scr2")
            nc.scalar.activation(out=scr2, in_=scr, func=AF.Exp, bias=0.0,
                                 scale=1.0)

            # ---- input loads (weights first; z1 needs wt+xqk only) ----
            wt = pin.tile([128, _WT_COLS], bf16, tag="wt")
            nc.sync.dma_start(out=wt, in_=wt_d)
            colf = pin.tile([128, 2], f32, tag="colf")
            nc.sync.dma_start(out=colf, in_=colf_d)
            xqk = pin.tile([128, _S], bf16, tag="xqk")
            nc.sync.dma_start(out=xqk, in_=xqk_d)
            misc = pin.tile([1, 128], bf16, tag="misc")
            nc.sync.dma_start(out=misc, in_=misc_d)
            vta = pin.tile([_D + 1, _S], bf16, tag="vta")
            nc.sync.dma_start(out=vta, in_=vta_d)

            w1 = wt[:, _W1:_W1 + 128]     # block-diag softplus'd layer-1 (T)
            w2 = wt[:, _W2:_W2 + 128]
            wsum2 = wt[:, _WSUM:_WSUM + 33]  # col sums of spW2q / spW2k
            eqek = wt[:, _EQEK:_EQEK + 33]   # q-mask col 0, k-mask col 32
            wv_aug = wt[0:_D + 1, _WV:_WV + _P]  # Wv.T rows 0-63, row 64 = 1
            wh_t = wt[0:_D, _WH:_WH + _P]        # Wh.T (partitions 0-63)
            wh_tk = wt[_D:2 * _D, _WH:_WH + _P]  # Wh.T (partitions 64-127)
            b1 = colf[:, 0:1]             # stacked layer-1 bias column
            negb2 = colf[:, 1:2]          # -b2 column (layer-2 Exp bias)
            ones_row = misc[0:1, 0:_P]       # [1,32] for the f_q broadcast
            ones128 = misc[0:1, 0:128]       # [1,128] for the -m2 rank-1s

            # ---- stacked ICNN layer 1 (q rows 0-63, k rows 64-127) ----
            z1_p = psA.tile([128, _S], f32, tag="big")
            nc.tensor.matmul(out=z1_p, lhsT=w1, rhs=xqk, start=True, stop=True)

            # phi_k chunks [t,r] / phi_q [r,s] fill the PE queue while the
            # scalar engine runs layer-1 softplus
            pk_p = psC.tile([128, NCH * _P], f32, tag="chunk")
            for c in range(NCH):
                nc.tensor.matmul(
                    out=pk_p[:, c * _P:(c + 1) * _P],
                    lhsT=xqk[_D:2 * _D, c * 128:(c + 1) * 128],
                    rhs=wh_tk,
                    start=True, stop=True,
                )
            phiq_p = psE.tile([_P, _S], f32, tag="phiq")
            nc.tensor.matmul(out=phiq_p, lhsT=wh_t, rhs=xqk[0:_D, :],
                             start=True, stop=True)

            # softplus layer 1: z1 = ln(1 + exp(z1_p + b1))
            e1 = pw.tile([128, _S], bf16, tag="e1")
            nc.scalar.activation(out=e1, in_=z1_p, func=AF.Exp, bias=b1,
                                 scale=1.0)
            z1 = pw.tile([128, _S], bf16, tag="z1")
            nc.scalar.activation(out=z1, in_=e1, func=AF.Ln, bias=1.0,
                                 scale=1.0)

            # ---- layer 2, clamp-free: z2 = x + ln(1+exp(-x)), x = pre+b2;
            # only the f/g row sums are needed downstream ----
            z2_p = psA.tile([128, _S], f32, tag="big")
            nc.tensor.matmul(out=z2_p, lhsT=w2, rhs=z1, start=True, stop=True)
            e2 = pw.tile([128, _S], bf16, tag="e2")
            nc.scalar.activation(out=e2, in_=z2_p, func=AF.Exp, bias=negb2,
                                 scale=-1.0)
            l2 = pw.tile([128, _S], bf16, tag="l2")
            nc.scalar.activation(out=l2, in_=e2, func=AF.Ln, bias=1.0,
                                 scale=1.0)

            # fg row 0 = f_q (sans sum(b2q)), row 32 = g_k (sans sum(b2k))
            fg_p = psB.tile([33, _S], f32, tag="mid")
            nc.tensor.matmul(out=fg_p, lhsT=wsum2, rhs=z1,
                             start=True, stop=False)
            nc.tensor.matmul(out=fg_p, lhsT=eqek, rhs=l2,
                             start=False, stop=True)

            fgs = pw.tile([33, _S], bf16, tag="fg_sb")
            nc.vector.tensor_copy(out=fgs, in_=fg_p)
            fq = fgs[0:1, :]
            # g_k into vta row 64 (pairs with wv_aug's 1.0 row)
            nc.vector.tensor_copy(out=vta[_D:_D + 1, :], in_=fgs[_P:_P + 1, :])

            # ---- cT[p,t] = u.T + g_k broadcast ; m2 = rowmax ----
            cT_p = psB.tile([_P, _S], f32, tag="mid")
            nc.tensor.matmul(out=cT_p, lhsT=wv_aug, rhs=vta[0:_D + 1, :],
                             start=True, stop=True)
            m2pad = pw.tile([_P, _P], f32, tag="m2pad")
            nc.vector.memset(m2pad, 0.0)
            nc.vector.reduce_max(m2pad[:, 0:1], cT_p, axis=AX)
            m2t = pw.tile([_P, _P], f32, tag="m2t")
            nc.vector.transpose(m2t, m2pad)  # row 0 of m2t = m2 as [1, P]
            negm2 = pw.tile([1, _P], bf16, tag="negm2")
            nc.vector.tensor_scalar_mul(negm2, m2t[0:1, 0:_P], -1.0)

            # ---- E chunks [t,p] = exp(u + g_k - m2) ----
            ec_p = psC.tile([128, NCH * _P], f32, tag="chunk")
            for c in range(NCH):
                nc.tensor.matmul(
                    out=ec_p[:, c * _P:(c + 1) * _P],
                    lhsT=vta[:, c * 128:(c + 1) * 128],
                    rhs=wv_aug,
                    start=True, stop=False,
                )
                nc.tensor.matmul(
                    out=ec_p[:, c * _P:(c + 1) * _P],
                    lhsT=ones128,
                    rhs=negm2,
                    start=False, stop=True,
                )

            # scalar queue: pk/phiq exps slot in after l2, before ec's exp
            pk = pw.tile([128, NCH * _P], bf16, tag="pk")
            nc.scalar.activation(out=pk, in_=pk_p, func=AF.Exp, bias=0.0,
                                 scale=1.0)
            phiq = pw.tile([_P, _S], bf16, tag="phiq_sb")
            nc.scalar.activation(out=phiq, in_=phiq_p, func=AF.Exp, bias=0.0,
                                 scale=1.0)
            ec = pw.tile([128, NCH * _P], bf16, tag="ec")
            nc.scalar.activation(out=ec, in_=ec_p, func=AF.Exp, bias=0.0,
                                 scale=1.0)

            # ---- M[r,p] = sum_t phi_k E ----
            M_p = psD.tile([_P, _P], f32, tag="M")
            for c in range(NCH):
                nc.tensor.matmul(
                    out=M_p,
                    lhsT=pk[:, c * _P:(c + 1) * _P],
                    rhs=ec[:, c * _P:(c + 1) * _P],
                    start=(c == 0), stop=(c == NCH - 1),
                )
            M_sb = pw.tile([_P, _P], bf16, tag="M_sb")
            nc.vector.tensor_copy(out=M_sb, in_=M_p)

            # f_q broadcast [p,s] (rank-1); m2 joins in the final fused add
            f_p = psA.tile([_P, _S], f32, tag="big")
            nc.tensor.matmul(out=f_p, lhsT=ones_row, rhs=fq,
                             start=True, stop=True)

            at_p = psA.tile([_P, _S], f32, tag="big")
            nc.tensor.matmul(out=at_p, lhsT=M_sb, rhs=phiq,
                             start=True, stop=True)

            lnA = pw.tile([_P, _S], f32, tag="lnA")
            nc.scalar.activation(out=lnA, in_=at_p, func=AF.Ln, bias=0.0,
                                 scale=1.0)
            # y^T = (lnA + m2[p]) + F
            from concourse.alu_op_type import AluOpType
            yT = pw.tile([_P, _S], f32, tag="yT")
            nc.vector.scalar_tensor_tensor(
                out=yT, in0=lnA, scalar=m2pad[:, 0:1], in1=f_p,
                op0=AluOpType.add, op1=AluOpType.add,
            )

            nc.sync.dma_start(out=y_d, in_=yT)

            if dump:
                for nm, t in [
                    ("d_z1", z1), ("d_l2", l2), ("d_fq", fq), ("d_pk", pk),
                    ("d_ec", ec), ("d_phiq", phiq), ("d_m2t", m2t),
                    ("d_Msb", M_sb), ("d_lnA", lnA),
                    ("d_vta64", vta[_D:_D + 1, :]), ("d_negm2", negm2),
                ]:
                    dd = nc.dram_tensor(nm, list(t.shape), t.dtype,
                                        kind="ExternalOutput").ap()
                    nc.sync.dma_start(out=dd, in_=t)

    if not nc.is_finalized():
        nc.finalize()
    return nc


def _host_inputs(q, k, v, spW1q, b1q, spW2q, b2q, spW1k, b1k, spW2k, b2k,
                 Wh, Wv):
    """Build the per-core input maps (numpy layout prep only)."""
    S, D, P = _S, _D, _P

    def z(shape):
        return np.zeros(shape, np.float32)

    wt = z((128, _WT_COLS))
    wt[0:D, _W1:_W1 + D] = spW1q.T
    wt[D:2 * D, _W1 + D:_W1 + 2 * D] = spW1k.T
    wt[0:D, _W2:_W2 + D] = spW2q.T
    wt[D:2 * D, _W2 + D:_W2 + 2 * D] = spW2k.T
    wt[0:D, _WSUM] = spW2q.sum(axis=0)      # wsumq[c] = sum_j spW2q[j,c]
    wt[D:2 * D, _WSUM + _P] = spW2k.sum(axis=0)  # g_k -> PSUM partition 32
    wt[0:D, _EQEK] = 1.0                    # eq
    wt[D:2 * D, _EQEK + _P] = 1.0           # ek
    wt[0:D, _WV:_WV + P] = Wv.T
    wt[D, _WV:_WV + P] = 1.0                # pairs with the g_k row of vta
    wt[0:D, _WH:_WH + P] = Wh.T
    wt[D:2 * D, _WH:_WH + P] = Wh.T         # copy at base partition 64 for pk
    colf = z((128, 2))
    colf[0:D, 0] = b1q
    colf[D:2 * D, 0] = b1k
    colf[0:D, 1] = -b2q
    colf[D:2 * D, 1] = -b2k
    misc = np.ones((1, 128), np.float32)

    import ml_dtypes

    def b(x):
        return np.asarray(x, np.float32).astype(ml_dtypes.bfloat16)

    wt_b = b(wt)
    misc_b = b(misc)
    in_maps = []
    for h in range(_H):
        xqk = z((128, S))
        xqk[0:D] = q[0, h].T
        xqk[D:2 * D] = k[0, h].T
        vta = z((D + 1, S))
        vta[0:D] = v[0, h].T
        # row D gets g_k on device
        in_maps.append(dict(wt=wt_b, colf=colf, xqk=b(xqk), vta=b(vta),
                            misc=misc_b))
    return in_maps


def kernel(**inputs):
    from concourse.bass_utils import run_bass_kernel_spmd

    np_in = {k: np.asarray(v) for k, v in inputs.items()}
    q, k, v = np_in["q"], np_in["k"], np_in["v"]

    def sp(x):  # softplus for the small weight matrices (host prep)
        return np.log1p(np.exp(x.astype(np.float64))).astype(np.float32)

    in_maps = _host_inputs(
        q, k, v,
        sp(np_in["sq_raw1"]), np_in["sq_b1"], sp(np_in["sq_raw2"]), np_in["sq_b2"],
        sp(np_in["sk_raw1"]), np_in["sk_b1"], sp(np_in["sk_raw2"]), np_in["sk_b2"],
        np_in["Wh"], np_in["Wv"],
    )
    # constant-in-t/s terms of y: sum(b2) linear parts and the -log(S)
    delta = (float(np_in["sq_b2"].sum()) + float(np_in["sk_b2"].sum())
             - _LN_S)

    if "nc" not in _CACHE:
        _CACHE["nc"] = _build_bass()
    nc = _CACHE["nc"]

    res = run_bass_kernel_spmd(nc, in_maps, list(range(_NCORES)))
    out = np.zeros((_B, _H, _S, _P), np.float32)
    for h in range(_H):
        out[0, h] = res.results[h]["y"].T + np.float32(delta)
    return out
